# revision 1
# baseline (speedup 1.0000x reference)
"""Trainium2 Bass kernel for a 6-layer transformer encoder (B=4, S=1024,
d_model=1024, 16 heads, d_ff=4096).

Sharding: token-parallel across 8 cores (B*S = 4096 tokens -> 512/core; each
core owns half of one batch element's sequence).  Per layer, one pair-wise
AllGather of the bf16-cast transposed activations lets each core rebuild K/V
for its full batch element; Q/attention-rows/O-proj/FFN/LayerNorms are
computed only for the core's own 512 tokens.

On-chip layout: activations are kept transposed (d_model on partitions,
tokens on free dim) so every projection/FFN matmul uses the natural [in,out]
weight as lhsT.  Matmuls run in bf16 with fp32 PSUM accumulation; the
residual stream and LayerNorm math stay fp32.  LayerNorm statistics are
ones-matmuls on the PE (which also broadcast them across partitions);
softmax is a fused exp(S/8 + mask_bias) activation, with denominators from a
ones-column appended to V, reciprocal'd in PSUM and partition-broadcast via a
small DRAM bounce.
"""

import sys
import os

for _p in ("/opt/trn_rl_repo", "/root/.axon_site/_ro/trn_rl_repo"):
    if os.path.isdir(_p) and _p not in sys.path:
        sys.path.insert(0, _p)

import numpy as np
import ml_dtypes

import concourse.bass as bass
import concourse.mybir as mybir
import concourse.tile as tile
from concourse.bass_utils import run_bass_kernel_spmd
from concourse.masks import make_identity
import concourse.bass_utils as _bu

# walrus's LDWEIGHTS fast-load path (FWL) is disabled by a hardcoded
# --enable-ldw-opt=false in bir_verify_and_optimise; with ~1 LDW per matmul
# that costs ~100ns each on the PE.  Flip it on for our compile.
if not getattr(_bu, "_ldw_patch", False):
    _orig_run_command = _bu.run_command

    def _run_command_ldw(cmd, *a, **kw):
        return _orig_run_command(cmd, *a, **kw)

    _bu.run_command = _run_command_ldw
    _bu._ldw_patch = True

VOCAB, D, H, DFF, L = 32000, 1024, 16, 4096, 6
B, S = 4, 1024
DK = D // H              # 64
NCORES = 8
TOK = (B * S) // NCORES  # 512 tokens per core
KT = D // 128            # 8
FT = DFF // 128          # 32
EPS = 1e-5
HT = TOK // 2            # 256, fc token-half

F32 = mybir.dt.float32
BF16 = mybir.dt.bfloat16
I32 = mybir.dt.int32
AF = mybir.ActivationFunctionType
OP = mybir.AluOpType

_NC = None
DEBUG_PROBES = False


def _build_nc():
    nc = bass.Bass("TRN2", target_bir_lowering=False, debug=False, num_devices=NCORES)

    emb = nc.dram_tensor("emb", [VOCAB, D], F32, kind="ExternalInput")
    src = nc.dram_tensor("src", [TOK, 1], I32, kind="ExternalInput")
    peT = nc.dram_tensor("peT", [D, TOK], F32, kind="ExternalInput")
    maskb = nc.dram_tensor("maskb", [128, KT], F32, kind="ExternalInput")
    koidx = nc.dram_tensor("koidx", [D, 1], I32, kind="ExternalInput")
    voidx = nc.dram_tensor("voidx", [TOK, 1], I32, kind="ExternalInput")
    wq = nc.dram_tensor("wq", [L, D, D], BF16, kind="ExternalInput")
    wk = nc.dram_tensor("wk", [L, D, D], BF16, kind="ExternalInput")
    wv = nc.dram_tensor("wv", [L, D, D], BF16, kind="ExternalInput")
    wo = nc.dram_tensor("wo", [L, D, D], BF16, kind="ExternalInput")
    w1 = nc.dram_tensor("w1", [L, D, DFF], BF16, kind="ExternalInput")
    w2 = nc.dram_tensor("w2", [L, DFF, D], BF16, kind="ExternalInput")
    bqT = nc.dram_tensor("bqT", [L, 128, KT], F32, kind="ExternalInput")
    bkT = nc.dram_tensor("bkT", [L, 128, KT], F32, kind="ExternalInput")
    boT = nc.dram_tensor("boT", [L, 128, KT], F32, kind="ExternalInput")
    b1T = nc.dram_tensor("b1T", [L, 128, FT], F32, kind="ExternalInput")
    b2T = nc.dram_tensor("b2T", [L, 128, KT], F32, kind="ExternalInput")
    g1T = nc.dram_tensor("g1T", [L, 128, KT], F32, kind="ExternalInput")
    be1T = nc.dram_tensor("be1T", [L, 128, KT], F32, kind="ExternalInput")
    g2T = nc.dram_tensor("g2T", [L, 128, KT], F32, kind="ExternalInput")
    be2T = nc.dram_tensor("be2T", [L, 128, KT], F32, kind="ExternalInput")
    xout = nc.dram_tensor("xout", [D, TOK], F32, kind="ExternalOutput")
    probes = {}
    if DEBUG_PROBES:
        for pn, shp, dt in [("p_ktl", [D, TOK], BF16), ("p_kto", [D, TOK], BF16),
                            ("p_qt", [D, TOK], BF16), ("p_vaug", [128, KT, H * 65], BF16),
                            ("p_attn", [D, TOK], BF16), ("p_recips", [H, TOK], F32),
                            ("p_x1", [D, TOK], F32)]:
            probes[pn] = nc.dram_tensor(pn, shp, dt, kind="ExternalOutput")

    with tile.TileContext(nc) as tc:
        with (
            tc.tile_pool(name="cpool", bufs=1) as cpool,
            tc.tile_pool(name="wp", bufs=2) as wp,
            tc.tile_pool(name="w2p", bufs=2) as w2p,
            tc.tile_pool(name="w1p", bufs=2) as w1p,
            tc.tile_pool(name="p1", bufs=1) as p1,
            tc.tile_pool(name="p2", bufs=2) as p2,
            tc.tile_pool(name="p3", bufs=3) as p3,
            tc.tile_pool(name="bp", bufs=8) as bp,
            tc.tile_pool(name="ps", bufs=8, space="PSUM") as ps,
            tc.tile_pool(name="dram", bufs=2, space="DRAM") as dram,
        ):
            _uid = [0]

            def _nm(tag):
                _uid[0] += 1
                return f"{tag}_{_uid[0]}"

            ident = cpool.tile([128, 128], BF16, tag="ident", name=_nm("ident"))
            make_identity(nc, ident[:])
            onesk = cpool.tile([128, 128], BF16, tag="onesk", name=_nm("onesk"))
            nc.vector.memset(onesk[:], 1.0 / D)
            maskb_sb = cpool.tile([128, KT], F32, tag="maskb", name=_nm("maskb"))
            nc.sync.dma_start(maskb_sb[:], maskb[:])
            eps_sb = cpool.tile([128, 1], F32, tag="eps", name=_nm("eps"))
            nc.vector.memset(eps_sb[:], EPS)

            def psum():
                return ps.tile([128, 512], F32, tag="ps", name=_nm("ps"))

            import contextlib

            def scope(name):
                return nc.named_scope(name)

            # ---------------- embedding ----------------
            peT_sb = p1.tile([128, KT, TOK], F32, tag="ht", name=_nm("ht"))  # reuse ht slot
            nc.sync.dma_start(peT_sb[:], peT.rearrange("(t p) n -> p t n", p=128))
            x_cur = p2.tile([128, KT, TOK], F32, tag="x", name=_nm("x"))
            for blk in range(TOK // 128):
                idx_t = p2.tile([128, 1], I32, tag="idx", name=_nm("idx"))
                nc.sync.dma_start(idx_t[:], src[blk * 128:(blk + 1) * 128, :])
                gat = p1.tile([128, D], F32, tag="vaug", name=_nm("vaug"))
                nc.gpsimd.indirect_dma_start(
                    out=gat[:], out_offset=None, in_=emb[:],
                    in_offset=bass.IndirectOffsetOnAxis(ap=idx_t[:, :1], axis=0),
                )
                gatb = p1.tile([128, D], BF16, tag="qt", name=_nm("qt"))
                nc.scalar.activation(gatb[:], gat[:], AF.Copy)
                for kt in range(KT):
                    tp = ps.tile([128, 128], BF16, tag="ps", name=_nm("ps"))
                    nc.tensor.transpose(tp[:], gatb[:, kt * 128:(kt + 1) * 128], ident[:])
                    nc.vector.scalar_tensor_tensor(
                        out=x_cur[:, kt, blk * 128:(blk + 1) * 128],
                        in0=tp[:], scalar=32.0,
                        in1=peT_sb[:, kt, blk * 128:(blk + 1) * 128],
                        op0=OP.mult, op1=OP.add,
                    )

            # ---------------- helpers ----------------
            def layer_norm(r, g_ap_of, be_ap_of, want_bf16):
                """r: [128, KT, TOK] f32 -> (xo f32, xb bf16|None)."""
                rb = p2.tile([128, KT, TOK], BF16, tag="bfs", name=_nm("bfs"))
                sq = p2.tile([128, KT, TOK], BF16, tag="bfs", name=_nm("bfs"))
                for kk in range(KT):
                    nc.vector.tensor_copy(rb[:, kk, :], r[:, kk, :])
                for kk in range(KT):
                    nc.vector.tensor_mul(sq[:, kk, :], rb[:, kk, :], rb[:, kk, :])
                pmu = psum()
                pm2 = psum()
                for kk in range(KT):
                    nc.tensor.matmul(pmu[:], onesk[:], rb[:, kk, :],
                                     start=(kk == 0), stop=(kk == KT - 1))
                for kk in range(KT):
                    nc.tensor.matmul(pm2[:], onesk[:], sq[:, kk, :],
                                     start=(kk == 0), stop=(kk == KT - 1))
                var = p2.tile([128, TOK], F32, tag="lns", name=_nm("lns"))
                nc.scalar.activation(var[:], pmu[:], AF.Square)
                nc.vector.tensor_sub(var[:], pm2[:], var[:])
                rstd = p2.tile([128, TOK], F32, tag="lns", name=_nm("lns"))
                nc.scalar.activation(rstd[:], var[:], AF.Sqrt, bias=eps_sb[:, 0:1])
                nc.vector.reciprocal(rstd[:], rstd[:])
                xo = p2.tile([128, KT, TOK], F32, tag="x", name=_nm("x"))
                xb = p2.tile([128, KT, TOK], BF16, tag="bfs", name=_nm("bfs")) if want_bf16 else None
                for kk in range(KT):
                    nc.vector.tensor_sub(xo[:, kk, :], r[:, kk, :], pmu[:])
                    nc.vector.tensor_mul(xo[:, kk, :], xo[:, kk, :], rstd[:])
                    nc.vector.tensor_scalar(
                        xo[:, kk, :], xo[:, kk, :], g_ap_of(kk), be_ap_of(kk),
                        OP.mult, OP.add)
                    if want_bf16:
                        nc.vector.tensor_copy(xb[:, kk, :], xo[:, kk, :])
                return xo, xb

            def load_bias8(t, l):
                b = bp.tile([128, KT], F32, tag="bias8", name=_nm("bias8"))
                nc.sync.dma_start(b[:], t[l])
                return b

            # ---------------- layers ----------------
            x_curb = None
            for l in range(L):
                # --- A: Q/K/V projections on the own shard; K/V pair-AllGather ---
                sc_ag = scope(f"L{l}.ag"); sc_ag.__enter__()
                if x_curb is None:
                    xcb = p2.tile([128, KT, TOK], BF16, tag="bfs", name=_nm("bfs"))
                    for kk in range(KT):
                        nc.vector.tensor_copy(xcb[:, kk, :], x_cur[:, kk, :])
                else:
                    xcb = x_curb
                sc_ag.__exit__(None, None, None)
                sc_at = scope(f"L{l}.attn"); sc_at.__enter__()
                bq_sb = load_bias8(bqT, l)
                bk_sb = load_bias8(bkT, l)
                # K^T for own tokens: [128, 8, TOK], then AllGather with the pair
                ktl = p1.tile([128, KT, TOK], BF16, tag="ktl", name=_nm("ktl"))
                kag_in = dram.tile([D, TOK], BF16, tag="kag_in", name=_nm("kag_in"))
                kag_out = dram.tile([2 * D, TOK], BF16, tag="kag_out", name=_nm("kag_out"))
                for half in range(2):
                    wkh = wp.tile([128, KT, 512], BF16, tag="wproj", name=_nm("wproj"))
                    nc.sync.dma_start(
                        wkh[:], wk[l, :, half * 512:(half + 1) * 512]
                        .rearrange("(t p) m -> p t m", p=128))
                    for m in range(4):
                        mg = half * 4 + m
                        pt = psum()
                        for kk in range(KT):
                            nc.tensor.matmul(
                                pt[:], wkh[:, kk, m * 128:(m + 1) * 128], xcb[:, kk, :],
                                start=(kk == 0), stop=(kk == KT - 1))
                        nc.vector.tensor_scalar_add(
                            ktl[:, mg, :], pt[:], bk_sb[:, mg:mg + 1])
                        nc.sync.dma_start(
                            kag_in[mg * 128:(mg + 1) * 128, :], ktl[:, mg, :])
                nc.gpsimd.collective_compute(
                    "AllGather", OP.bypass,
                    ins=[kag_in[:]], outs=[kag_out[:]],
                    replica_groups=[[2 * i, 2 * i + 1] for i in range(NCORES // 2)],
                )
                # V for own tokens: [tok, dv] = [128, 4, D], then AllGather
                vl = p1.tile([128, 4, D], BF16, tag="vl", name=_nm("vl"))
                vag_in = dram.tile([TOK, D], BF16, tag="vag_in", name=_nm("vag_in"))
                vag_out = dram.tile([2 * TOK, D], BF16, tag="vag_out", name=_nm("vag_out"))
                for half in range(2):
                    wvh = wp.tile([128, KT, 512], BF16, tag="wproj", name=_nm("wproj"))
                    nc.sync.dma_start(
                        wvh[:], wv[l, :, half * 512:(half + 1) * 512]
                        .rearrange("(t p) m -> p t m", p=128))
                    for mt in range(4):   # own token tiles
                        pt = psum()
                        for kk in range(KT):
                            nc.tensor.matmul(
                                pt[:], xcb[:, kk, mt * 128:(mt + 1) * 128],
                                wvh[:, kk, :],
                                start=(kk == 0), stop=(kk == KT - 1))
                        nc.vector.tensor_copy(
                            vl[:, mt, half * 512:(half + 1) * 512], pt[:])
                        nc.sync.dma_start(
                            vag_in[mt * 128:(mt + 1) * 128, half * 512:(half + 1) * 512],
                            vl[:, mt, half * 512:(half + 1) * 512])
                nc.gpsimd.collective_compute(
                    "AllGather", OP.bypass,
                    ins=[vag_in[:]], outs=[vag_out[:]],
                    replica_groups=[[2 * i, 2 * i + 1] for i in range(NCORES // 2)],
                )
                # Q^T for own tokens
                qt = p1.tile([128, KT, TOK], BF16, tag="qt", name=_nm("qt"))
                for half in range(2):
                    wqh = wp.tile([128, KT, 512], BF16, tag="wproj", name=_nm("wproj"))
                    nc.sync.dma_start(
                        wqh[:], wq[l, :, half * 512:(half + 1) * 512]
                        .rearrange("(t p) m -> p t m", p=128))
                    for m in range(4):
                        mg = half * 4 + m
                        pt = psum()
                        for kk in range(KT):
                            nc.tensor.matmul(
                                pt[:], wqh[:, kk, m * 128:(m + 1) * 128], xcb[:, kk, :],
                                start=(kk == 0), stop=(kk == KT - 1))
                        nc.vector.tensor_scalar_add(
                            qt[:, mg, :], pt[:], bq_sb[:, mg:mg + 1])
                # pair's K^T rows gathered by per-core row indices (k-order: own|pair)
                kto = p1.tile([128, KT, TOK], BF16, tag="ht", name=_nm("ht"))
                for g in range(KT):
                    kidx = bp.tile([128, 1], I32, tag="koidx", name=_nm("koidx"))
                    nc.sync.dma_start(kidx[:], koidx[g * 128:(g + 1) * 128, :])
                    nc.gpsimd.indirect_dma_start(
                        out=kto[:, g, :], out_offset=None, in_=kag_out[:],
                        in_offset=bass.IndirectOffsetOnAxis(ap=kidx[:, :1], axis=0),
                    )
                # V augmented with ones cols: [128, 8, 16*65]; kt 0..3 own, 4..7 pair
                vaug = p1.tile([128, KT, H * 65], BF16, tag="vaug", name=_nm("vaug"))
                nc.vector.memset(
                    vaug[:].rearrange("p t (h x) -> p t h x", x=65)[:, :, :, 64:65], 1.0)
                for mt in range(4):
                    nc.vector.tensor_copy(
                        vaug[:, mt, :].rearrange("p (h x) -> p h x", x=65)[:, :, 0:64],
                        vl[:, mt, :].rearrange("p (h c) -> p h c", c=64))
                for mt in range(4):
                    vidx = bp.tile([128, 1], I32, tag="voidx", name=_nm("voidx"))
                    nc.sync.dma_start(vidx[:], voidx[mt * 128:(mt + 1) * 128, :])
                    vstg = p1.tile([128, D], BF16, tag="vstg", name=_nm("vstg"))
                    nc.gpsimd.indirect_dma_start(
                        out=vstg[:], out_offset=None, in_=vag_out[:],
                        in_offset=bass.IndirectOffsetOnAxis(ap=vidx[:, :1], axis=0),
                    )
                    nc.vector.tensor_copy(
                        vaug[:, 4 + mt, :].rearrange("p (h x) -> p h x", x=65)[:, :, 0:64],
                        vstg[:].rearrange("p (h c) -> p h c", c=64))
                recips = dram.tile([H, TOK], F32, tag="recips", name=_nm("recips"))
                attn = p1.tile([128, KT, TOK], BF16, tag="attn", name=_nm("attn"))

                def kt_lhs(kt, mj, prow):
                    # k-order own|pair: kt 0..3 from local K^T, 4..7 from gathered
                    if kt < 4:
                        return ktl[prow:prow + 64, mj, kt * 128:(kt + 1) * 128]
                    return kto[prow:prow + 64, mj, (kt - 4) * 128:(kt - 3) * 128]

                # --- attention per head ---
                for h in range(H):
                    prow = (h % 2) * 64
                    mj = h // 2
                    exps_h = [p3.tile([128, 4, TOK], BF16, tag="exps", name=_nm("exps"))
                              for _ in range(2)]
                    for kt in range(KT):
                        pt = psum()
                        nc.tensor.matmul(
                            pt[:], kt_lhs(kt, mj, prow),
                            qt[prow:prow + 64, mj, :],
                            start=True, stop=True)
                        nc.scalar.activation(
                            exps_h[kt // 4][:, kt % 4, :], pt[:], AF.Exp,
                            scale=DK ** -0.5, bias=maskb_sb[:, kt:kt + 1])
                    pav = psum()
                    for kt in range(KT):
                        vo = 65 * h
                        nc.tensor.matmul(
                            pav[0:65, :], vaug[:, kt, vo:vo + 65],
                            exps_h[kt // 4][:, kt % 4, :],
                            start=(kt == 0), stop=(kt == KT - 1))
                    # denominator -> reciprocal -> SBUF row -> DRAM
                    recrow = p1.tile([65, TOK], F32, tag="recrow", name=_nm("recrow"))
                    nc.vector.reciprocal(recrow[64:65, :], pav[64:65, :])
                    nc.sync.dma_start(recips[h:h + 1, :], recrow[64:65, :])
                    if h % 2 == 0:
                        nc.scalar.activation(
                            attn[0:64, mj, :], pav[0:64, :], AF.Copy)
                    else:
                        stg = p1.tile([64, TOK], BF16, tag="stage", name=_nm("stage"))
                        nc.scalar.activation(stg[:], pav[0:64, :], AF.Copy)
                        nc.sync.dma_start(attn[64:128, mj, :], stg[:])
                if DEBUG_PROBES and l == 0:
                    nc.sync.dma_start(probes["p_ktl"].rearrange("(t p) n -> p t n", p=128), ktl[:])
                    nc.sync.dma_start(probes["p_kto"].rearrange("(t p) n -> p t n", p=128), kto[:])
                    nc.sync.dma_start(probes["p_qt"].rearrange("(t p) n -> p t n", p=128), qt[:])
                    nc.sync.dma_start(probes["p_vaug"][:], vaug[:])
                    nc.sync.dma_start(probes["p_attn"].rearrange("(t p) n -> p t n", p=128), attn[:])
                    nc.sync.dma_start(probes["p_recips"][:], recips[:])
                sc_at.__exit__(None, None, None)
                sc_o = scope(f"L{l}.o_ln1"); sc_o.__enter__()
                # --- D: normalize attn rows, O-proj, residual, LN1 ---
                for t in range(KT):
                    rbc = p2.tile([128, TOK], F32, tag="rbc", name=_nm("rbc"))
                    nc.sync.dma_start(
                        rbc[:],
                        recips[2 * t:2 * t + 2, None, :].to_broadcast((2, 64, TOK)))
                    nc.vector.tensor_mul(attn[:, t, :], attn[:, t, :], rbc[:])
                bo_sb = load_bias8(boT, l)
                r1 = p2.tile([128, KT, TOK], F32, tag="x", name=_nm("x"))
                for half in range(2):
                    woh = wp.tile([128, KT, 512], BF16, tag="wproj", name=_nm("wproj"))
                    nc.sync.dma_start(
                        woh[:], wo[l, :, half * 512:(half + 1) * 512]
                        .rearrange("(t p) m -> p t m", p=128))
                    for m in range(4):
                        mg = half * 4 + m
                        pt = psum()
                        for kk in range(KT):
                            nc.tensor.matmul(
                                pt[:], woh[:, kk, m * 128:(m + 1) * 128], attn[:, kk, :],
                                start=(kk == 0), stop=(kk == KT - 1))
                        nc.vector.scalar_tensor_tensor(
                            out=r1[:, mg, :], in0=pt[:],
                            scalar=bo_sb[:, mg:mg + 1], in1=x_cur[:, mg, :],
                            op0=OP.add, op1=OP.add)
                g1_sb = load_bias8(g1T, l)
                be1_sb = load_bias8(be1T, l)
                x1, x1b = layer_norm(
                    r1, lambda kk: g1_sb[:, kk:kk + 1], lambda kk: be1_sb[:, kk:kk + 1],
                    want_bf16=True)
                if DEBUG_PROBES and l == 0:
                    nc.sync.dma_start(probes["p_x1"].rearrange("(t p) n -> p t n", p=128), x1[:])

                sc_o.__exit__(None, None, None)
                sc_f = scope(f"L{l}.ffn"); sc_f.__enter__()
                # --- E: FFN (token-halved) ---
                b1_sb = bp.tile([128, FT], F32, tag="bias32", name=_nm("bias32"))
                nc.sync.dma_start(b1_sb[:], b1T[l])
                b2_sb = load_bias8(b2T, l)
                r2 = p2.tile([128, KT, TOK], F32, tag="x", name=_nm("x"))
                ht = p1.tile([128, FT, TOK], BF16, tag="ht", name=_nm("ht"))
                for e in range(8):   # w1 eighths: dff cols e*512..
                    w1e = w1p.tile([128, KT, 512], BF16, tag="w1e", name=_nm("w1e"))
                    nc.sync.dma_start(
                        w1e[:], w1[l, :, e * 512:(e + 1) * 512]
                        .rearrange("(t p) m -> p t m", p=128))
                    for m in range(4):
                        fm = e * 4 + m
                        pt = psum()
                        for kk in range(KT):
                            nc.tensor.matmul(
                                pt[:], w1e[:, kk, m * 128:(m + 1) * 128],
                                x1b[:, kk, :],
                                start=(kk == 0), stop=(kk == KT - 1))
                        nc.scalar.activation(
                            ht[:, fm, :], pt[:], AF.Relu,
                            bias=b1_sb[:, fm:fm + 1])
                fps = [psum() for _ in range(KT)]
                for kk in range(FT):
                    w2c = w2p.tile([128, D], BF16, tag="w2c", name=_nm("w2c"))
                    nc.sync.dma_start(w2c[:], w2[l, kk * 128:(kk + 1) * 128, :])
                    for m in range(KT):
                        nc.tensor.matmul(
                            fps[m][:], w2c[:, m * 128:(m + 1) * 128],
                            ht[:, kk, :],
                            start=(kk == 0), stop=(kk == FT - 1))
                for m in range(KT):
                    nc.vector.scalar_tensor_tensor(
                        out=r2[:, m, :], in0=fps[m][:],
                        scalar=b2_sb[:, m:m + 1],
                        in1=x1[:, m, :],
                        op0=OP.add, op1=OP.add)

                sc_f.__exit__(None, None, None)
                sc_l2 = scope(f"L{l}.ln2"); sc_l2.__enter__()
                g2_sb = load_bias8(g2T, l)
                be2_sb = load_bias8(be2T, l)
                x_cur, x_curb = layer_norm(
                    r2, lambda kk: g2_sb[:, kk:kk + 1], lambda kk: be2_sb[:, kk:kk + 1],
                    want_bf16=True)
                sc_l2.__exit__(None, None, None)

            nc.sync.dma_start(
                xout.rearrange("(t p) n -> p t n", p=128), x_cur[:])

    return nc


MAXW = 1


def split_wait_overflow(nc, maxw=MAXW):
    """walrus in this toolchain rejects instructions with more than one sem
    wait; split excess waits onto preceding NoOp carriers on the same engine."""
    for f in nc.m.functions:
        for bb in f.blocks:
            if not any(i.sync_info and len(i.sync_info.on_wait) > maxw
                       for i in bb.instructions):
                continue
            newlist = []
            for inst in bb.instructions:
                si = inst.sync_info
                if si and len(si.on_wait) > maxw:
                    waits = list(si.on_wait)
                    extra, keep = waits[:-maxw], waits[-maxw:]
                    for i in range(0, len(extra), maxw):
                        newlist.append(mybir.InstNoOp(
                            name=f"{inst.name}-ws{i}", opcode="NoOp",
                            engine=inst.engine, debug=inst.debug, ins=[], outs=[],
                            sync_info=mybir.SyncInfo(
                                on_wait=extra[i:i + maxw], on_update=[]),
                        ))
                    inst.sync_info = mybir.SyncInfo(
                        on_wait=keep, on_update=list(si.on_update))
                newlist.append(inst)
            bb.instructions = newlist


def _get_nc():
    global _NC
    if _NC is None:
        _NC = _build_nc()
        split_wait_overflow(_NC)
    return _NC


def _to_bf16(a):
    return np.asarray(a, dtype=np.float32).astype(ml_dtypes.bfloat16)


def _bias_t(v, kt=KT):
    # [L, d] -> [L, 128, d//128] with column t = v[:, 128t:128t+128]
    v = np.asarray(v, dtype=np.float32)
    return np.ascontiguousarray(v.reshape(L, kt, 128).transpose(0, 2, 1))


def kernel(**inputs):
    nc = _get_nc()

    src = np.asarray(inputs["src"]).astype(np.int32).reshape(-1)      # [4096]
    src_mask = np.asarray(inputs["src_mask"]).astype(np.float32)      # [B,1,1,S]
    emb = np.asarray(inputs["emb"], dtype=np.float32)
    pe = np.asarray(inputs["pe"], dtype=np.float32)
    shared = {
        "emb": emb,
        "wq": _to_bf16(inputs["wq"]), "wk": _to_bf16(inputs["wk"]),
        "wv": _to_bf16(inputs["wv"]), "wo": _to_bf16(inputs["wo"]),
        "w1": _to_bf16(inputs["w1"]), "w2": _to_bf16(inputs["w2"]),
        "bqT": _bias_t(inputs["bq"]), "bkT": _bias_t(inputs["bk"]),
        "b1T": _bias_t(inputs["b1"], FT), "b2T": _bias_t(inputs["b2"]),
        "g1T": _bias_t(inputs["g1"]), "be1T": _bias_t(inputs["be1"]),
        "g2T": _bias_t(inputs["g2"]), "be2T": _bias_t(inputs["be2"]),
    }
    # fold the V bias through the O projection: attn rows sum to 1, so
    # out = attn@(V + bv) @ wo + bo = attn@V@wo + (bv@wo + bo)
    wo_f = np.asarray(inputs["wo"], dtype=np.float32)
    bv_f = np.asarray(inputs["bv"], dtype=np.float32)
    bo_f = np.asarray(inputs["bo"], dtype=np.float32)
    bo_eff = np.stack([bo_f[l] + bv_f[l] @ wo_f[l] for l in range(L)])
    shared["boT"] = _bias_t(bo_eff)

    in_maps = []
    for c in range(NCORES):
        b = c // 2
        half = c % 2
        m = dict(shared)
        m["src"] = np.ascontiguousarray(
            src[c * TOK:(c + 1) * TOK].reshape(TOK, 1))
        m["peT"] = np.ascontiguousarray(
            pe[half * TOK:half * TOK + TOK, :D].T.astype(np.float32))
        mb = (src_mask[b, 0, 0, :] - 1.0) * 1e9
        own = slice(half * TOK, half * TOK + TOK)
        pair = slice((1 - half) * TOK, (1 - half) * TOK + TOK)
        mb_perm = np.concatenate([mb[own], mb[pair]])
        m["maskb"] = np.ascontiguousarray(
            mb_perm.reshape(KT, 128).T.astype(np.float32))
        o = 1 - half  # pair-local rank of the partner
        m["koidx"] = np.ascontiguousarray(
            (np.arange(D, dtype=np.int32) + o * D).reshape(D, 1))
        m["voidx"] = np.ascontiguousarray(
            (np.arange(TOK, dtype=np.int32) + o * TOK).reshape(TOK, 1))
        in_maps.append(m)

    res = run_bass_kernel_spmd(nc, in_maps, list(range(NCORES)))
    out = np.empty((B * S, D), dtype=np.float32)
    for c in range(NCORES):
        out[c * TOK:(c + 1) * TOK] = res.results[c]["xout"].T
    return out.reshape(B, S, D)

